# revision 1
# baseline (speedup 1.0000x reference)
"""Trainium2 Bass kernel: complex nearest-neighbor 2x2 upsampling.

y[b, i, j, c] = complex(x_re, x_im)[b, i//2, j//2, c]
  inputs : x_re, x_im  f32 [16, 128, 128, 64]
  output : complex64   [16, 256, 256, 64]

Strategy (data-parallel over batch, 2 examples per core on 8 cores):
  - partition dim = h (128 rows), loop over (b, w-chunk)
  - DMA in re/im w-chunks with large contiguous lines (8 KiB/partition)
  - compute engines (DVE for re, ACT for im) write the complex-interleaved,
    w-duplicated row block into SBUF (broadcast AP duplicates along w)
  - two output DMAs per chunk (row duplication i=2h / 2h+1), each writing
    fully contiguous 32 KiB lines per partition
All device work is pure data movement: per core 16 MiB in + 64 MiB out.
"""
import numpy as np

import concourse.bass as bass
import concourse.tile as tile
from concourse import bacc, mybir
from concourse import bass_utils

# Full-problem constants (hardcoded per harness contract)
B, H, W, C = 16, 128, 128, 64
N_CORES = 8
B_SHARD = B // N_CORES  # 2 examples per core
WC = 32  # w-chunk size

_CACHE = {}


def build_nc(wc=WC, inp_bufs=2, out_bufs=3):
    """Build and compile the per-core Bass module (B_SHARD examples)."""
    nc = bacc.Bacc("TRN2", debug=False, num_devices=N_CORES)
    x_re = nc.dram_tensor(
        "x_re", [B_SHARD, H, W, C], mybir.dt.float32, kind="ExternalInput"
    ).ap()
    x_im = nc.dram_tensor(
        "x_im", [B_SHARD, H, W, C], mybir.dt.float32, kind="ExternalInput"
    ).ap()
    # f32 view of the complex64 output: last dim is (c, comp) interleaved
    y = nc.dram_tensor(
        "y", [B_SHARD, 2 * H, 2 * W, 2 * C], mybir.dt.float32, kind="ExternalOutput"
    ).ap()

    with tile.TileContext(nc) as tc:
        with (
            tc.tile_pool(name="inp", bufs=inp_bufs) as inp,
            tc.tile_pool(name="out", bufs=out_bufs) as outp,
        ):
            for b in range(B_SHARD):
                for wi in range(W // wc):
                    re_t = inp.tile([H, wc * C], mybir.dt.float32, tag="re")
                    nc.sync.dma_start(
                        re_t[:],
                        x_re[b, :, wi * wc:(wi + 1) * wc, :].rearrange(
                            "h w c -> h (w c)"
                        ),
                    )
                    im_t = inp.tile([H, wc * C], mybir.dt.float32, tag="im")
                    nc.sync.dma_start(
                        im_t[:],
                        x_im[b, :, wi * wc:(wi + 1) * wc, :].rearrange(
                            "h w c -> h (w c)"
                        ),
                    )
                    cplx = outp.tile([H, wc * 2 * C * 2], mybir.dt.float32, tag="cplx")
                    dst5 = cplx[:].rearrange(
                        "p (w dup c comp) -> p w dup c comp", w=wc, dup=2, c=C, comp=2
                    )
                    src_re = (
                        re_t[:]
                        .rearrange("p (w c) -> p w c", w=wc)
                        .unsqueeze(2)
                        .broadcast_to([H, wc, 2, C])
                    )
                    src_im = (
                        im_t[:]
                        .rearrange("p (w c) -> p w c", w=wc)
                        .unsqueeze(2)
                        .broadcast_to([H, wc, 2, C])
                    )
                    nc.vector.tensor_copy(dst5[:, :, :, :, 0], src_re)
                    nc.scalar.copy(dst5[:, :, :, :, 1], src_im)
                    for r in range(2):
                        nc.sync.dma_start(
                            y[b, r::2, 2 * wi * wc:2 * (wi + 1) * wc, :].rearrange(
                                "i j cc -> i (j cc)"
                            ),
                            cplx[:],
                        )
    nc.compile()
    return nc


def _get_nc():
    if "nc" not in _CACHE:
        _CACHE["nc"] = build_nc()
    return _CACHE["nc"]


def run_sharded(x_re, x_im, trace=False):
    """Run the SPMD kernel; returns (full complex64 output, BassKernelResults)."""
    nc = _get_nc()
    in_maps = [
        {
            "x_re": np.ascontiguousarray(x_re[m * B_SHARD:(m + 1) * B_SHARD]),
            "x_im": np.ascontiguousarray(x_im[m * B_SHARD:(m + 1) * B_SHARD]),
        }
        for m in range(N_CORES)
    ]
    res = bass_utils.run_bass_kernel_spmd(
        nc, in_maps, core_ids=list(range(N_CORES)), trace=trace
    )
    parts = [res.results[m]["y"] for m in range(N_CORES)]
    out_f32 = np.concatenate(parts, axis=0)  # [16, 256, 256, 128] f32
    out = out_f32.view(np.complex64)  # [16, 256, 256, 64] c64
    return out, res


def kernel(x_re, x_im):
    x_re = np.asarray(x_re, dtype=np.float32)
    x_im = np.asarray(x_im, dtype=np.float32)
    out, _ = run_sharded(x_re, x_im, trace=False)
    return out


# revision 3
# speedup vs baseline: 1.0311x; 1.0311x over previous
"""Trainium2 Bass kernel: complex nearest-neighbor 2x2 upsampling.

y[b, i, j, c] = complex(x_re, x_im)[b, i//2, j//2, c]
  inputs : x_re, x_im  f32 [16, 128, 128, 64]
  output : complex64   [16, 256, 256, 64]

Data-parallel over batch: 2 examples per core on 8 cores. Per core the kernel
is pure data movement (16 MiB in + 64 MiB out):
  - partition dim = h (128 rows)
  - load full-example re/im planes with 4 MiB DMAs (32 KiB/partition lines)
    on the ACT HWDGE ring
  - DVE (re) + ACT (im) copies build the complex-interleaved, w-duplicated
    rows in SBUF (broadcast APs do the duplication)
  - stores on the Sync HWDGE ring write fully contiguous 64 KiB/partition
    lines; row duplication (i = 2h, 2h+1) comes from storing each tile twice
    (or once with a 0-stride repeat AP)
"""
import numpy as np

import concourse.bass as bass
import concourse.tile as tile
from concourse import bacc, mybir
from concourse import bass_utils

# Full-problem constants (hardcoded per harness contract)
B, H, W, C = 16, 128, 128, 64
N_CORES = 8
B_SHARD = B // N_CORES  # 2 examples per core

_CACHE = {}

# default configuration (best measured: ~207us/core uncontended, ~245us median
# under full 8-core HBM contention; chip roofline for 8x(16MiB in + 64MiB out)
# at ~2.9TB/s is ~234us)
CFG = dict(wc=32, full_b_loads=True, load_engine="gpsimd", store_repeat=False,
           inp_bufs=2, out_bufs=2)


def build_nc(cfg=None):
    """Build and compile the per-core Bass module (B_SHARD examples)."""
    cfg = {**CFG, **(cfg or {})}
    wc = cfg["wc"]
    nc = bacc.Bacc("TRN2", debug=False, num_devices=N_CORES)
    x_re = nc.dram_tensor(
        "x_re", [B_SHARD, H, W, C], mybir.dt.float32, kind="ExternalInput"
    ).ap()
    x_im = nc.dram_tensor(
        "x_im", [B_SHARD, H, W, C], mybir.dt.float32, kind="ExternalInput"
    ).ap()
    # f32 view of the complex64 output: last dim is (c, comp) interleaved
    y = nc.dram_tensor(
        "y", [B_SHARD, 2 * H, 2 * W, 2 * C], mybir.dt.float32, kind="ExternalOutput"
    ).ap()

    load = getattr(nc, cfg["load_engine"]).dma_start

    with tile.TileContext(nc) as tc:
        with (
            tc.tile_pool(name="inp", bufs=cfg["inp_bufs"]) as inp,
            tc.tile_pool(name="outp", bufs=cfg["out_bufs"]) as outp,
        ):
            for b in range(B_SHARD):
                if cfg["full_b_loads"]:
                    re_t = inp.tile([H, W * C], mybir.dt.float32, tag="re")
                    load(re_t[:], x_re[b].rearrange("h w c -> h (w c)"))
                    im_t = inp.tile([H, W * C], mybir.dt.float32, tag="im")
                    load(im_t[:], x_im[b].rearrange("h w c -> h (w c)"))
                for wi in range(W // wc):
                    if not cfg["full_b_loads"]:
                        re_t = inp.tile([H, wc * C], mybir.dt.float32, tag="re")
                        load(re_t[:], x_re[b, :, wi * wc:(wi + 1) * wc, :]
                             .rearrange("h w c -> h (w c)"))
                        im_t = inp.tile([H, wc * C], mybir.dt.float32, tag="im")
                        load(im_t[:], x_im[b, :, wi * wc:(wi + 1) * wc, :]
                             .rearrange("h w c -> h (w c)"))
                        sl = slice(0, wc * C)
                    else:
                        sl = slice(wi * wc * C, (wi + 1) * wc * C)
                    cplx = outp.tile([H, wc * 2 * C * 2], mybir.dt.float32, tag="cplx")
                    dst5 = cplx[:].rearrange(
                        "p (w dup c comp) -> p w dup c comp", w=wc, dup=2, c=C, comp=2
                    )
                    src_re = (re_t[:, sl].rearrange("p (w c) -> p w c", w=wc)
                              .unsqueeze(2).broadcast_to([H, wc, 2, C]))
                    src_im = (im_t[:, sl].rearrange("p (w c) -> p w c", w=wc)
                              .unsqueeze(2).broadcast_to([H, wc, 2, C]))
                    nc.vector.tensor_copy(dst5[:, :, :, :, 0], src_re)
                    nc.scalar.copy(dst5[:, :, :, :, 1], src_im)
                    if cfg["store_repeat"]:
                        dst = y[b, :, 2 * wi * wc:2 * (wi + 1) * wc, :].rearrange(
                            "(h r) j cc -> h r (j cc)", r=2
                        )
                        src = cplx[:].unsqueeze(1).broadcast_to(
                            [H, 2, wc * 2 * C * 2]
                        )
                        nc.sync.dma_start(dst, src)
                    else:
                        for r in range(2):
                            nc.sync.dma_start(
                                y[b, r::2, 2 * wi * wc:2 * (wi + 1) * wc, :]
                                .rearrange("i j cc -> i (j cc)"),
                                cplx[:],
                            )
    nc.compile()
    return nc


def _get_nc(cfg=None):
    key = tuple(sorted({**CFG, **(cfg or {})}.items()))
    if key not in _CACHE:
        _CACHE[key] = build_nc(cfg)
    return _CACHE[key]


def run_sharded(x_re, x_im, trace=False, cfg=None):
    """Run the SPMD kernel; returns (full complex64 output, BassKernelResults)."""
    nc = _get_nc(cfg)
    in_maps = [
        {
            "x_re": np.ascontiguousarray(x_re[m * B_SHARD:(m + 1) * B_SHARD]),
            "x_im": np.ascontiguousarray(x_im[m * B_SHARD:(m + 1) * B_SHARD]),
        }
        for m in range(N_CORES)
    ]
    res = bass_utils.run_bass_kernel_spmd(
        nc, in_maps, core_ids=list(range(N_CORES)), trace=trace
    )
    parts = [res.results[m]["y"] for m in range(N_CORES)]
    out_f32 = np.concatenate(parts, axis=0)  # [16, 256, 256, 128] f32
    out = out_f32.view(np.complex64)  # [16, 256, 256, 64] c64
    return out, res


def kernel(x_re, x_im):
    x_re = np.asarray(x_re, dtype=np.float32)
    x_im = np.asarray(x_im, dtype=np.float32)
    out, _ = run_sharded(x_re, x_im, trace=False)
    return out
